# revision 3
# baseline (speedup 1.0000x reference)
"""MoE BaseLayer kernel for Trainium2 (8 NeuronCores, expert parallelism).

Strategy (per the expert-parallelism sharding hint):
  * Host computes token->expert assignment (scores = x @ centroids.T, argmax)
    -- this IS the shard function: tokens are dispatched to the core owning
    their expert (the host-side equivalent of the All2All in the original).
  * Core e holds expert e's weights only and runs the BaseSublayer
    (LayerNorm -> FF1 -> ReLU -> FF2 -> residual) plus the
    alpha = sigmoid(x . centroid) blend for its routed tokens.
  * Host scatters per-core outputs back to original token order (combine).

Device kernel layout (per core, C padded routed tokens):
  * token-major tiles [128 tok, D] for LN stats / alpha / residual blend
  * PE transpose -> x^T [D, C] tiles as FF1 moving operand
  * FF1 computes H^T [F, C] (w1 stationary), bias+ReLU on ACT with
    per-partition bias
  * FF2 contracts over F (h^T stationary, w2 moving) -> token-major [tok, D]
  * matmuls run as float32r (TRN2 fast-FP32 path, 4x the fp32 rate)
"""

import numpy as np

E, D, F = 8, 512, 2048
LN_EPS = 1e-5
P = 128

_CACHE = {}


def _build(C, mm_dtype_name="float32r"):
    import concourse.tile as tile
    from concourse import bacc, mybir
    from concourse.masks import make_identity

    f32 = mybir.dt.float32
    mmdt = getattr(mybir.dt, mm_dtype_name)
    NT = C // P           # token tiles
    KT = D // P           # contraction tiles over D (4)
    FT = F // P           # F tiles (16)
    NG = (NT + 3) // 4    # groups of <=512 tokens (PSUM bank limit)

    nc = bacc.Bacc("TRN2", target_bir_lowering=False, num_devices=E)
    xs_d = nc.dram_tensor("xs", [C, D], f32, kind="ExternalInput")
    w1_d = nc.dram_tensor("w1", [D, F], f32, kind="ExternalInput")
    w2_d = nc.dram_tensor("w2", [F, D], f32, kind="ExternalInput")
    b1_d = nc.dram_tensor("b1t", [P, FT], f32, kind="ExternalInput")
    gb_d = nc.dram_tensor("gb", [P, D], f32, kind="ExternalInput")
    bb_d = nc.dram_tensor("bb", [P, D], f32, kind="ExternalInput")
    b2b_d = nc.dram_tensor("b2b", [P, D], f32, kind="ExternalInput")
    cb_d = nc.dram_tensor("cb", [P, D], f32, kind="ExternalInput")
    y_d = nc.dram_tensor("y", [C, D], f32, kind="ExternalOutput")

    with tile.TileContext(nc) as tc:
        with (
            tc.tile_pool(name="consts", bufs=1) as consts,
            tc.tile_pool(name="wpool", bufs=1) as wpool,
            tc.tile_pool(name="xpool", bufs=1) as xpool,
            tc.tile_pool(name="hpool", bufs=2) as hpool,
            tc.tile_pool(name="spool", bufs=4) as spool,
            tc.tile_pool(name="opool", bufs=3) as opool,
            tc.tile_pool(name="pt", bufs=2, space="PSUM") as pt,
            tc.tile_pool(name="pf1", bufs=2, space="PSUM") as pf1,
            tc.tile_pool(name="pf2", bufs=2, space="PSUM") as pf2,
        ):
            ident = consts.tile([P, P], f32, name="ident", tag="ident")
            make_identity(nc, ident)
            b1T = consts.tile([P, FT], f32, name="b1T", tag="b1T")
            nc.sync.dma_start(out=b1T, in_=b1_d[:])
            gb = consts.tile([P, D], f32, name="gb", tag="gb")
            nc.sync.dma_start(out=gb, in_=gb_d[:])
            bb = consts.tile([P, D], f32, name="bb", tag="bb")
            nc.sync.dma_start(out=bb, in_=bb_d[:])
            b2b = consts.tile([P, D], f32, name="b2b", tag="b2b")
            nc.sync.dma_start(out=b2b, in_=b2b_d[:])
            cb = consts.tile([P, D], f32, name="cb", tag="cb")
            nc.sync.dma_start(out=cb, in_=cb_d[:])
            eps_t = consts.tile([P, 1], f32, name="eps_t", tag="eps")
            nc.vector.memset(eps_t, LN_EPS)

            # token-major input tiles
            xs_t = []
            for i in range(NT):
                t = xpool.tile([P, D], f32, name=f"xs{i}", tag=f"xs{i}")
                nc.sync.dma_start(out=t, in_=xs_d[i * P:(i + 1) * P, :])
                xs_t.append(t)

            # weights: w1 as KT x (F/512) chunks [128, 512]; w2 as FT tiles [128, D]
            w1s = {}
            for kt in range(KT):
                for g in range(FT // 4):
                    t = wpool.tile([P, 512], mmdt, name=f"w1_{kt}_{g}", tag=f"w1_{kt}_{g}")
                    nc.sync.dma_start(
                        out=t,
                        in_=w1_d[kt * P:(kt + 1) * P, g * 512:(g + 1) * 512].bitcast(mmdt),
                    )
                    w1s[(kt, g)] = t
            w2s = []
            for ft in range(FT):
                t = wpool.tile([P, D], mmdt, name=f"w2_{ft}", tag=f"w2_{ft}")
                nc.sync.dma_start(out=t, in_=w2_d[ft * P:(ft + 1) * P, :].bitcast(mmdt))
                w2s.append(t)

            alphas = [None] * NT
            for grp in range(NG):
                t0 = grp * 4                      # first token tile of group
                tn = min(4, NT - t0)              # tiles in this group
                Cg = tn * P

                # LayerNorm (token-major) + alpha + transpose to [D, Cg]
                xlnT = [
                    hpool.tile([P, Cg], mmdt, name=f"xlnT{kt}", tag=f"xlnT{kt}")
                    for kt in range(KT)
                ]
                for i in range(t0, t0 + tn):
                    xi = xs_t[i]
                    stats = spool.tile([P, 6], f32, name="stats", tag="stats")
                    nc.vector.bn_stats(out=stats, in_=xi)
                    mv = spool.tile([P, 2], f32, name="mv", tag="mv")
                    nc.vector.bn_aggr(out=mv, in_=stats)
                    rs = spool.tile([P, 1], f32, name="rs", tag="rs")
                    nc.scalar.activation(
                        out=rs, in_=mv[:, 1:2],
                        func=mybir.ActivationFunctionType.Sqrt,
                        bias=eps_t, scale=1.0,
                    )
                    nc.vector.reciprocal(out=rs, in_=rs)
                    xln = spool.tile([P, D], f32, name="xln", tag="xln")
                    nc.vector.tensor_scalar(
                        out=xln, in0=xi,
                        scalar1=mv[:, 0:1], scalar2=rs,
                        op0=mybir.AluOpType.subtract, op1=mybir.AluOpType.mult,
                    )
                    nc.vector.tensor_mul(out=xln, in0=xln, in1=gb)
                    nc.vector.tensor_add(out=xln, in0=xln, in1=bb)

                    # alpha = sigmoid(x . centroid)
                    prod = spool.tile([P, D], f32, name="prod", tag="prod")
                    nc.vector.tensor_mul(out=prod, in0=xi, in1=cb)
                    dot = spool.tile([P, 1], f32, name="dot", tag="dot")
                    nc.vector.reduce_sum(
                        out=dot, in_=prod, axis=mybir.AxisListType.X
                    )
                    al = spool.tile([P, 1], f32, name=f"alpha{i}", tag=f"alpha{i}")
                    nc.scalar.activation(
                        out=al, in_=dot,
                        func=mybir.ActivationFunctionType.Sigmoid,
                    )
                    alphas[i] = al

                    # transpose LN output into xlnT
                    col = (i - t0) * P
                    for kt in range(KT):
                        ps = pt.tile([P, P], f32, name="ps_t", tag="ps_t")
                        nc.tensor.transpose(
                            ps, xln[:, kt * P:(kt + 1) * P], ident
                        )
                        nc.vector.tensor_copy(
                            out=xlnT[kt][:, col:col + P], in_=ps
                        )

                # FF1: H^T [F, Cg] = w1^T @ xln^T ; bias + relu
                h_t = []
                for ft in range(FT):
                    acc = pf1.tile([P, Cg], f32, name="acc1", tag="acc1")
                    for kt in range(KT):
                        lhsT = w1s[(kt, ft // 4)][:, (ft % 4) * P:(ft % 4 + 1) * P]
                        nc.tensor.matmul(
                            acc,
                            lhsT,
                            xlnT[kt][:],
                            start=(kt == 0), stop=(kt == KT - 1),
                        )
                    h = hpool.tile([P, Cg], mmdt, name=f"h{ft}", tag=f"h{ft}")
                    nc.scalar.activation(
                        out=h, in_=acc,
                        func=mybir.ActivationFunctionType.Relu,
                        bias=b1T[:, ft:ft + 1], scale=1.0,
                    )
                    h_t.append(h)

                # FF2 + blend: y = x + alpha * (h @ w2 + b2)
                for i in range(t0, t0 + tn):
                    col = (i - t0) * P
                    acc = pf2.tile([P, D], f32, name="acc2", tag="acc2")
                    for ft in range(FT):
                        nc.tensor.matmul(
                            acc,
                            h_t[ft][:, col:col + P],
                            w2s[ft][:],
                            start=(ft == 0), stop=(ft == FT - 1),
                        )
                    yo = opool.tile([P, D], f32, name="yo", tag="yo")
                    nc.vector.tensor_add(out=yo, in0=acc, in1=b2b)
                    nc.vector.tensor_scalar_mul(out=yo, in0=yo, scalar1=alphas[i])
                    nc.vector.tensor_add(out=yo, in0=yo, in1=xs_t[i])
                    nc.sync.dma_start(out=y_d[i * P:(i + 1) * P, :], in_=yo)

    nc.compile()
    return nc


def _get_nc(C):
    if C not in _CACHE:
        _CACHE[C] = _build(C)
    return _CACHE[C]


def kernel(x, centroids, ln_g, ln_b, W1, b1, W2, b2):
    from concourse.bass_utils import run_bass_kernel_spmd

    x = np.asarray(x)
    orig_shape = x.shape
    feats = np.ascontiguousarray(x.reshape(-1, D), dtype=np.float32)
    T = feats.shape[0]
    centroids = np.asarray(centroids, dtype=np.float32)

    # Routing on host = the shard function (host-side All2All dispatch).
    # Use jax-on-CPU so score numerics match the reference bit-for-bit
    # (argmax near-ties resolve identically); fall back to numpy.
    try:
        import jax
        import jax.numpy as jnp

        with jax.default_device(jax.devices("cpu")[0]):
            scores = jnp.asarray(feats) @ jnp.asarray(centroids).T
            assign = np.asarray(jnp.argmax(scores, axis=1))
    except Exception:
        assign = np.argmax(feats @ centroids.T, axis=1)

    idx = [np.nonzero(assign == e)[0] for e in range(E)]
    max_count = max(len(ix) for ix in idx)
    C = max(256, -(-max_count // P) * P)

    nc = _get_nc(C)

    in_maps = []
    for e in range(E):
        xs = np.zeros((C, D), dtype=np.float32)
        xs[: len(idx[e])] = feats[idx[e]]
        in_maps.append(
            dict(
                xs=xs,
                w1=np.ascontiguousarray(W1[e], dtype=np.float32),
                w2=np.ascontiguousarray(W2[e], dtype=np.float32),
                b1t=np.ascontiguousarray(
                    np.asarray(b1[e], dtype=np.float32).reshape(F // P, P).T
                ),
                gb=np.broadcast_to(
                    np.asarray(ln_g[e], dtype=np.float32), (P, D)
                ).copy(),
                bb=np.broadcast_to(
                    np.asarray(ln_b[e], dtype=np.float32), (P, D)
                ).copy(),
                b2b=np.broadcast_to(
                    np.asarray(b2[e], dtype=np.float32), (P, D)
                ).copy(),
                cb=np.broadcast_to(centroids[e], (P, D)).copy(),
            )
        )

    res = run_bass_kernel_spmd(nc, in_maps, core_ids=list(range(E)))

    out = np.empty((T, D), dtype=np.float32)
    for e in range(E):
        out[idx[e]] = res.results[e]["y"][: len(idx[e])]
    return out.reshape(orig_shape)


# revision 7
# speedup vs baseline: 1.0250x; 1.0250x over previous
"""MoE BaseLayer kernel for Trainium2 (8 NeuronCores, expert parallelism).

Strategy (per the expert-parallelism sharding hint):
  * Host computes token->expert assignment (scores = x @ centroids.T, argmax)
    -- this IS the shard function: tokens are dispatched to the core owning
    their expert (the host-side equivalent of the All2All in the original),
    and the gate alpha = sigmoid(score of the assigned expert) falls out of
    the same routing scores.
  * Core e holds expert e's weights only and runs the BaseSublayer
    (LayerNorm -> FF1 -> ReLU -> FF2 -> residual) + alpha blend for its
    routed tokens. LayerNorm's affine (ln_g, ln_b) is folded into W1/b1 on
    the host (exact reparameterization): relu(((x-mu)*rs*g + b) @ W1 + b1)
    == relu((x-mu)*rs @ (g*W1) + (b@W1 + b1)).
  * Host scatters per-core outputs back to original token order (combine).

Device kernel layout (per core, C padded routed tokens):
  * token-major tiles [128 tok, D] for LN stats / residual blend
  * PE transpose -> xhat^T [D, C] tiles as FF1 moving operand
  * FF1 computes H^T [F-tile, C] (w1 stationary), bias+ReLU on ACT with
    per-partition bias; FF2 (h^T stationary, w2 moving) is interleaved per
    F-tile, accumulating token-major [tok, D] PSUM tiles
  * matmuls run as float32r (TRN2 fast-FP32 path, 4x the fp32 rate)
  * weight DMAs are issued in consumption order (w1 column-group, then the
    matching w2 tiles) so the PE starts ~5us in and streams behind DMA
"""

import numpy as np

E, D, F = 8, 512, 2048
LN_EPS = 1e-5
P = 128

_CACHE = {}


def _build(C, mm_dtype_name="float32r"):
    import concourse.tile as tile
    from concourse import bacc, mybir
    from concourse.masks import make_identity

    f32 = mybir.dt.float32
    mmdt = getattr(mybir.dt, mm_dtype_name)
    NT = C // P           # token tiles
    KT = D // P           # contraction tiles over D (4)
    FT = F // P           # F tiles (16)
    NG = (NT + 3) // 4    # groups of <=512 tokens (PSUM bank limit)

    nc = bacc.Bacc("TRN2", target_bir_lowering=False, num_devices=E)
    xs_d = nc.dram_tensor("xs", [C, D], f32, kind="ExternalInput")
    w1_d = nc.dram_tensor("w1", [D, F], f32, kind="ExternalInput")
    w2_d = nc.dram_tensor("w2", [F, D], f32, kind="ExternalInput")
    b1_d = nc.dram_tensor("b1t", [P, FT], f32, kind="ExternalInput")
    b2b_d = nc.dram_tensor("b2b", [P, D], f32, kind="ExternalInput")
    al_d = nc.dram_tensor("alpha", [C, 1], f32, kind="ExternalInput")
    y_d = nc.dram_tensor("y", [C, D], f32, kind="ExternalOutput")

    with tile.TileContext(nc) as tc:
        with (
            tc.tile_pool(name="consts", bufs=1) as consts,
            tc.tile_pool(name="wpool", bufs=1) as wpool,
            tc.tile_pool(name="xpool", bufs=1) as xpool,
            tc.tile_pool(name="hpool", bufs=3) as hpool,
            tc.tile_pool(name="spool", bufs=4) as spool,
            tc.tile_pool(name="opool", bufs=3) as opool,
            tc.tile_pool(name="pt", bufs=2, space="PSUM") as pt,
            tc.tile_pool(name="pf1", bufs=2, space="PSUM") as pf1,
            tc.tile_pool(name="pf2", bufs=1, space="PSUM") as pf2,
        ):
            ident = consts.tile([P, P], f32, name="ident", tag="ident")
            make_identity(nc, ident)

            # token-major input tiles + small consts first
            xs_t = []
            for i in range(NT):
                t = xpool.tile([P, D], f32, name=f"xs{i}", tag=f"xs{i}")
                nc.sync.dma_start(out=t, in_=xs_d[i * P:(i + 1) * P, :])
                xs_t.append(t)
            b1T = consts.tile([P, FT], f32, name="b1T", tag="b1T")
            nc.sync.dma_start(out=b1T, in_=b1_d[:])
            alT = []
            for i in range(NT):
                t = consts.tile([P, 1], f32, name=f"al{i}", tag=f"al{i}")
                nc.sync.dma_start(out=t, in_=al_d[i * P:(i + 1) * P, :])
                alT.append(t)
            b2b = consts.tile([P, D], f32, name="b2b", tag="b2b")
            nc.sync.dma_start(out=b2b, in_=b2b_d[:])
            eps_t = consts.tile([P, 1], f32, name="eps_t", tag="eps")
            nc.vector.memset(eps_t, LN_EPS)

            # weights, issued in FF consumption order:
            # per column-group g: w1[:, g*512:(g+1)*512] (all 4 K tiles),
            # then w2 tiles ft = 4g .. 4g+3
            w1s = {}
            w2s = {}
            for g in range(FT // 4):
                for kt in range(KT):
                    t = wpool.tile(
                        [P, 512], mmdt, name=f"w1_{kt}_{g}", tag=f"w1_{kt}_{g}"
                    )
                    nc.sync.dma_start(
                        out=t,
                        in_=w1_d[kt * P:(kt + 1) * P, g * 512:(g + 1) * 512].bitcast(mmdt),
                    )
                    w1s[(kt, g)] = t
                for ft in range(4 * g, 4 * g + 4):
                    t = wpool.tile([P, D], mmdt, name=f"w2_{ft}", tag=f"w2_{ft}")
                    nc.sync.dma_start(
                        out=t, in_=w2_d[ft * P:(ft + 1) * P, :].bitcast(mmdt)
                    )
                    w2s[ft] = t

            for grp in range(NG):
                t0 = grp * 4                      # first token tile of group
                tn = min(4, NT - t0)              # tiles in this group
                Cg = tn * P

                # LayerNorm stats (token-major): batch by op to avoid ACT
                # table thrash, then normalize and transpose to [D, Cg]
                mvs = []
                for i in range(t0, t0 + tn):
                    stats = spool.tile([P, 6], f32, name="stats", tag="stats")
                    nc.vector.bn_stats(out=stats, in_=xs_t[i])
                    mv = spool.tile([P, 2], f32, name="mv", tag=f"mv{i - t0}")
                    nc.vector.bn_aggr(out=mv, in_=stats)
                    mvs.append(mv)
                rss = []
                for i in range(t0, t0 + tn):
                    rs = spool.tile([P, 1], f32, name="rs", tag=f"rs{i - t0}")
                    nc.scalar.activation(
                        out=rs, in_=mvs[i - t0][:, 1:2],
                        func=mybir.ActivationFunctionType.Sqrt,
                        bias=eps_t, scale=1.0,
                    )
                    rss.append(rs)
                for i in range(t0, t0 + tn):
                    nc.vector.reciprocal(out=rss[i - t0], in_=rss[i - t0])

                xlnT = [
                    hpool.tile([P, Cg], mmdt, name=f"xlnT{kt}", tag=f"xlnT{kt}")
                    for kt in range(KT)
                ]
                for i in range(t0, t0 + tn):
                    xln = spool.tile([P, D], f32, name="xln", tag="xln")
                    nc.vector.tensor_scalar(
                        out=xln, in0=xs_t[i],
                        scalar1=mvs[i - t0][:, 0:1], scalar2=rss[i - t0],
                        op0=mybir.AluOpType.subtract, op1=mybir.AluOpType.mult,
                    )
                    col = (i - t0) * P
                    for kt in range(KT):
                        ps = pt.tile([P, P], f32, name="ps_t", tag="ps_t")
                        nc.tensor.transpose(
                            ps, xln[:, kt * P:(kt + 1) * P], ident
                        )
                        nc.vector.tensor_copy(
                            out=xlnT[kt][:, col:col + P], in_=ps
                        )

                # FF1 + FF2 pipelined per F-tile:
                #   H^T[ft] = relu(w1[:,ft].T @ xhat^T + b1[ft])
                #   yacc[i] += H^T[ft][:, tok_i].T @ w2[ft]
                yaccs = [
                    pf2.tile([P, D], f32, name=f"yacc{i - t0}", tag=f"yacc{i - t0}")
                    for i in range(t0, t0 + tn)
                ]
                for ft in range(FT):
                    acc = pf1.tile([P, Cg], f32, name="acc1", tag="acc1")
                    for kt in range(KT):
                        lhsT = w1s[(kt, ft // 4)][:, (ft % 4) * P:(ft % 4 + 1) * P]
                        nc.tensor.matmul(
                            acc,
                            lhsT,
                            xlnT[kt][:],
                            start=(kt == 0), stop=(kt == KT - 1),
                        )
                    h = hpool.tile([P, Cg], mmdt, name="h", tag="h")
                    nc.scalar.activation(
                        out=h, in_=acc,
                        func=mybir.ActivationFunctionType.Relu,
                        bias=b1T[:, ft:ft + 1], scale=1.0,
                    )
                    for i in range(t0, t0 + tn):
                        col = (i - t0) * P
                        nc.tensor.matmul(
                            yaccs[i - t0],
                            h[:, col:col + P],
                            w2s[ft][:],
                            start=(ft == 0), stop=(ft == FT - 1),
                        )

                # blend: y = x + alpha * (yacc + b2)
                for i in range(t0, t0 + tn):
                    yo = opool.tile([P, D], f32, name="yo", tag="yo")
                    nc.vector.tensor_add(out=yo, in0=yaccs[i - t0], in1=b2b)
                    nc.vector.tensor_scalar_mul(out=yo, in0=yo, scalar1=alT[i])
                    nc.vector.tensor_add(out=yo, in0=yo, in1=xs_t[i])
                    nc.sync.dma_start(out=y_d[i * P:(i + 1) * P, :], in_=yo)

    nc.compile()
    return nc


def _get_nc(C):
    if C not in _CACHE:
        _CACHE[C] = _build(C)
    return _CACHE[C]


def _route(feats, centroids):
    """Token->expert assignment + gate, computed the same way the reference
    does (jax on CPU) so argmax near-ties resolve identically."""
    try:
        import jax
        import jax.numpy as jnp

        with jax.default_device(jax.devices("cpu")[0]):
            scores = jnp.asarray(feats) @ jnp.asarray(centroids).T
            assign = jnp.argmax(scores, axis=1)
            alpha = jax.nn.sigmoid(
                jnp.take_along_axis(scores, assign[:, None], axis=1)
            )
            return np.asarray(assign), np.asarray(alpha, dtype=np.float32)
    except Exception:
        scores = feats @ centroids.T
        assign = np.argmax(scores, axis=1)
        alpha = 1.0 / (1.0 + np.exp(-scores[np.arange(len(assign)), assign]))
        return assign, alpha[:, None].astype(np.float32)


def prepare(x, centroids, ln_g, ln_b, W1, b1, W2, b2):
    """Shard the full inputs: route tokens to experts, build per-core input
    maps. Returns (C, in_maps, idx, orig_shape)."""
    x = np.asarray(x)
    orig_shape = x.shape
    feats = np.ascontiguousarray(x.reshape(-1, D), dtype=np.float32)
    centroids = np.asarray(centroids, dtype=np.float32)

    assign, alpha = _route(feats, centroids)

    idx = [np.nonzero(assign == e)[0] for e in range(E)]
    max_count = max(len(ix) for ix in idx)
    C = max(256, -(-max_count // P) * P)

    W1 = np.asarray(W1, dtype=np.float32)
    W2 = np.asarray(W2, dtype=np.float32)
    b1 = np.asarray(b1, dtype=np.float32)
    b2 = np.asarray(b2, dtype=np.float32)
    ln_g = np.asarray(ln_g, dtype=np.float32)
    ln_b = np.asarray(ln_b, dtype=np.float32)

    in_maps = []
    for e in range(E):
        xs = np.zeros((C, D), dtype=np.float32)
        xs[: len(idx[e])] = feats[idx[e]]
        al = np.zeros((C, 1), dtype=np.float32)
        al[: len(idx[e])] = alpha[idx[e]]
        # fold LN affine into the first FFN layer (exact reparameterization)
        w1_eff = ln_g[e][:, None] * W1[e]
        b1_eff = ln_b[e] @ W1[e] + b1[e]
        in_maps.append(
            dict(
                xs=xs,
                w1=np.ascontiguousarray(w1_eff),
                w2=np.ascontiguousarray(W2[e]),
                b1t=np.ascontiguousarray(b1_eff.reshape(F // P, P).T),
                b2b=np.broadcast_to(b2[e], (P, D)).copy(),
                alpha=al,
            )
        )
    return C, in_maps, idx, orig_shape


def kernel(x, centroids, ln_g, ln_b, W1, b1, W2, b2):
    from concourse.bass_utils import run_bass_kernel_spmd

    C, in_maps, idx, orig_shape = prepare(
        x, centroids, ln_g, ln_b, W1, b1, W2, b2
    )
    nc = _get_nc(C)
    res = run_bass_kernel_spmd(nc, in_maps, core_ids=list(range(E)))

    T = int(np.prod(orig_shape[:-1]))
    out = np.empty((T, D), dtype=np.float32)
    for e in range(E):
        out[idx[e]] = res.results[e]["y"][: len(idx[e])]
    return out.reshape(orig_shape)
